# revision 52
# baseline (speedup 1.0000x reference)
"""Trainium2 Bass kernel for EnhancedReconstructionLoss (0.8*MSE + 0.2*SSIM-loss).

Sharding: pure data parallel. Batch 32 -> 8 cores x 4 images (12 planes of
512x512 each). Each core computes partial sums (sum x^2, sum y^2, sum x*y,
sum ssim_map*9); host combines into the scalar loss.

Per-core pipeline per 512x512 plane (inputs pre-cast to bf16 on host; all
reductions accumulate in fp32 on-chip):
  - load x,y as a [128, 5, 512] plane tensor of row-shifted tiles
    (tile t holds rows 128t-1..128t+126) so cross-tile vertical-filter edge
    matmuls only need base-partition-0 operands
  - xx=x^2, yy=y^2 (ScalarE, fp32 accum for MSE), xy=x*y (accum), zz=xx+yy
    computed on the whole plane at once
  - vertical 3-tap box filter via TensorE banded matmul -> PSUM (fp32)
  - PSUM->SBUF bf16 copy (hw allows only one PSUM operand per instruction),
    horizontal 3-tap via two shifted-AP adds (DVE, bf16 2x mode)
  - SSIM pointwise tail once per plane at FD=2048 using tensor_tensor (2x)
    and tensor_scalar (4x) ops with the 1/9 pool normalizations folded into
    constants; the host multiplies the ssim sum by 9 at the end
"""

import sys
import numpy as np

for _p in ("/opt/trn_rl_repo", "/root/.axon_site/_ro/trn_rl_repo"):
    if _p not in sys.path:
        sys.path.insert(0, _p)

N_CORES = 8
IMG = 512
PLANES = 12          # 4 images x 3 channels per core
# Tiles are shifted by -1 row: tile t = rows 128t-1..128t+126 (tile 0 only
# 127 rows, tile 4 only row 511). Cross-tile matmul edges then only ever
# need the FIRST rows of the next tile (base partition 0, a hw requirement).
TILE_ROWS = [(0, 127), (127, 255), (255, 383), (383, 511), (511, 512)]
NT = 5
NCHUNK = 4
C1 = 0.01 ** 2
C2 = 0.03 ** 2
EPS = 1e-8

CFG = {
    "dma_eng": "sync",
}

_compiled = None


def _build_nc():
    from contextlib import ExitStack
    import concourse.bass as bass
    import concourse.tile as tile
    from concourse import bacc, mybir

    f32 = mybir.dt.float32
    bf16 = mybir.dt.bfloat16
    Alu = mybir.AluOpType
    Act = mybir.ActivationFunctionType

    nc = bacc.Bacc("TRN2", target_bir_lowering=False, debug=False,
                   enable_asserts=True, num_devices=N_CORES)
    x_d = nc.dram_tensor("x", [PLANES, IMG, IMG], bf16, kind="ExternalInput").ap()
    y_d = nc.dram_tensor("y", [PLANES, IMG, IMG], bf16, kind="ExternalInput").ap()
    band_d = nc.dram_tensor("band", [257, 128], bf16, kind="ExternalInput").ap()
    out_d = nc.dram_tensor("out", [128, 4], f32, kind="ExternalOutput").ap()

    dma = getattr(nc, CFG["dma_eng"])

    with tile.TileContext(nc) as tc, ExitStack() as ctx:
        consts = ctx.enter_context(tc.tile_pool(name="consts", bufs=1))
        inp = ctx.enter_context(tc.tile_pool(name="inp", bufs=3))
        pre = ctx.enter_context(tc.tile_pool(name="pre", bufs=2))
        psum = ctx.enter_context(tc.tile_pool(name="psum", bufs=2, space="PSUM"))
        taps = ctx.enter_context(tc.tile_pool(name="taps", bufs=3))
        s2p = ctx.enter_context(tc.tile_pool(name="s2p", bufs=2))
        tail = ctx.enter_context(tc.tile_pool(name="tail", bufs=2))
        trec = ctx.enter_context(tc.tile_pool(name="trec", bufs=1))
        tshort = ctx.enter_context(tc.tile_pool(name="tshort", bufs=8))
        accs = ctx.enter_context(tc.tile_pool(name="accs", bufs=1))

        band_a = consts.tile([128, 128], bf16, tag="band_a")  # i-j in {0,1,2}
        dma.dma_start(out=band_a, in_=band_d[0:128, :])
        band_b = consts.tile([127, 128], bf16, tag="band_b")  # i-j in {-1,0,1}
        dma.dma_start(out=band_b, in_=band_d[128:255, :])
        e2 = consts.tile([2, 128], bf16, tag="e2")
        dma.dma_start(out=e2, in_=band_d[255:257, :])
        e1 = consts.tile([1, 128], bf16, tag="e1")
        dma.dma_start(out=e1, in_=band_d[256:257, :])

        nacc = 5 + 2 * (PLANES - 1)
        xxacc = accs.tile([128, nacc], f32, tag="xxacc")
        yyacc = accs.tile([128, nacc], f32, tag="yyacc")
        xyacc = accs.tile([128, nacc], f32, tag="xyacc")
        ssacc = accs.tile([128, PLANES], f32, tag="ssacc")
        for a in (xxacc, yyacc, xyacc, ssacc):
            nc.vector.memset(a, 0.0)

        def load_plane(dst, src_d, p):
            # tile 0: rows 0..126 at partitions 0..126
            dma.dma_start(out=dst[0:127, 0, :], in_=src_d[p, 0:127, :])
            # tiles 1..2: rows 127..382, partition p = row 128t-1+p
            mid = src_d[p, 127:383, :].rearrange("(t r) c -> r t c", r=128)
            dma.dma_start(out=dst[:, 1:3, :], in_=mid)
            # tile 3: rows 383..510
            dma.dma_start(out=dst[:, 3, :], in_=src_d[p, 383:511, :])
            # tile 4: row 511 at partition 0
            dma.dma_start(out=dst[0:1, 4, :], in_=src_d[p, 511:512, :])

        for p in range(PLANES):
            # ---- load plane + pre-pool pointwise on the whole plane ----
            xp = inp.tile([128, NT, IMG], bf16, tag="xp")
            yp = inp.tile([128, NT, IMG], bf16, tag="yp")
            if p < 3:  # = inp pool bufs: zero each slot once
                # zero this pool slot once before its first loads: the pad
                # regions (t0 partition 127, t4 partitions 1..127) are never
                # DMA'd, and slot values persist across the bufs=2 rotation,
                # so derived tensors inherit exact zeros there
                nc.gpsimd.memset(xp, 0.0)
                nc.gpsimd.memset(yp, 0.0)
            load_plane(xp, x_d, p)
            load_plane(yp, y_d, p)

            # pre-pool in two tile-halves so chunk 0/1 matmuls can start
            # before the last tiles of the plane have landed
            xxp = pre.tile([128, NT, IMG], bf16, tag="xx")
            yyp = pre.tile([128, NT, IMG], bf16, tag="yy")
            xyp = pre.tile([128, NT, IMG], bf16, tag="xy")
            zzp = pre.tile([128, NT, IMG], bf16, tag="zz")
            # plane 0 at per-tile granularity so compute starts as soon as
            # the first tile lands; later planes in two halves (less per-op
            # overhead, prefetch already hides the latency)
            if p == 0:
                hsplit = ((0, 1), (1, 2), (2, 3), (3, 4), (4, 5))
            else:
                hsplit = ((0, 3), (3, 5))
            for h, (t0, t1) in enumerate(hsplit):
                g = h if p == 0 else 5 + 2 * (p - 1) + h
                sl = (slice(None), slice(t0, t1), slice(None))
                nc.scalar.activation(xxp[sl], xp[sl], Act.Square,
                                     accum_out=xxacc[:, g:g + 1])
                nc.scalar.activation(yyp[sl], yp[sl], Act.Square,
                                     accum_out=yyacc[:, g:g + 1])
                nc.vector.scalar_tensor_tensor(
                    out=xyp[sl], in0=xp[sl], scalar=1.0, in1=yp[sl],
                    op0=Alu.mult, op1=Alu.mult, accum_out=xyacc[:, g:g + 1])
                nc.gpsimd.tensor_add(zzp[sl], xxp[sl], yyp[sl])

            streams = [xp, yp, zzp, xyp]

            # S2 holds the fully box-filtered sums for the whole plane:
            # [partition, stream, chunk, col]
            S2 = s2p.tile([128, 4, NCHUNK, IMG], bf16, tag="S2")

            # ---- per output chunk: vertical matmul + horizontal taps ----
            for c in range(NCHUNK):
                V = psum.tile([128, 4, IMG], f32, tag="V")
                for s, st in enumerate(streams):
                    main_band = band_b if c == 0 else band_a
                    main_rhs = st[0:127, 0, :] if c == 0 else st[:, c, :]
                    if c < NCHUNK - 1:
                        edge = (e2[0:2, :], st[0:2, c + 1, :])
                    else:
                        edge = (e1[0:1, :], st[0:1, c + 1, :])
                    mms = [(main_band, main_rhs), edge]
                    for i, (lhsT, rhs) in enumerate(mms):
                        nc.tensor.matmul(V[:, s, :], lhsT, rhs,
                                         start=(i == 0), stop=(i == len(mms) - 1))

                Vs = taps.tile([128, 4, IMG], bf16, tag="Vs")
                nc.scalar.activation(Vs, V, Act.Copy)

                A = taps.tile([128, 4, IMG], bf16, tag="A")
                nc.vector.memset(A[:, :, 0:1], 0.0)
                nc.vector.tensor_add(A[:, :, 1:IMG], Vs[:, :, 0:IMG - 1],
                                     Vs[:, :, 1:IMG])
                nc.vector.tensor_add(S2[:, :, c, 0:IMG - 1], A[:, :, 0:IMG - 1],
                                     Vs[:, :, 1:IMG])
                nc.scalar.activation(S2[:, :, c, IMG - 1:IMG],
                                     A[:, :, IMG - 1:IMG], Act.Copy)

            # ---- SSIM pointwise tail, whole plane at once (FD = 2048) ----
            # With S = 9*mu (raw 3x3 box sums):
            #   num1 = 2*P/81 + C1            (P = Sx*Sy)
            #   num2 = 2*Sxy/9 + C2 - (num1 - C1)
            #   den1 = qsum/81 + C1           (qsum = Sx^2 + Sy^2)
            #   den2' = 9*den2 = Szz - (qsum/9 - 9*C2)
            #   ssim = num1*num2 / (den1*den2) = 9 * num / dd,  dd = den1*den2'
            # (the x9 is applied on the host)
            FD = NCHUNK * IMG
            Sx = S2[:, 0, :, :]
            Sy = S2[:, 1, :, :]
            Szz = S2[:, 2, :, :]
            Sxy = S2[:, 3, :, :]
            qx = tshort.tile([128, FD], bf16, tag="ts")
            nc.scalar.activation(qx, Sx, Act.Square)
            qy = tshort.tile([128, FD], bf16, tag="ts")
            nc.scalar.activation(qy, Sy, Act.Square)
            qsum = tshort.tile([128, FD], bf16, tag="ts")
            nc.vector.tensor_add(qsum, qx, qy)
            den1 = tail.tile([128, FD], bf16, tag="den1")
            nc.vector.tensor_scalar(out=den1, in0=qsum, scalar1=1.0 / 81.0,
                                    scalar2=C1, op0=Alu.mult, op1=Alu.add)
            U3 = tshort.tile([128, FD], bf16, tag="ts")
            nc.vector.tensor_scalar(out=U3, in0=qsum, scalar1=1.0 / 9.0,
                                    scalar2=-9.0 * C2, op0=Alu.mult, op1=Alu.add)
            den2 = tail.tile([128, FD], bf16, tag="den2")
            nc.vector.tensor_sub(den2, Szz, U3)
            P = tshort.tile([128, FD], bf16, tag="ts")
            nc.vector.tensor_mul(P, Sx, Sy)
            num1 = tail.tile([128, FD], bf16, tag="num1")
            nc.vector.tensor_scalar(out=num1, in0=P, scalar1=2.0 / 81.0,
                                    scalar2=C1, op0=Alu.mult, op1=Alu.add)
            t1 = tshort.tile([128, FD], bf16, tag="ts")
            nc.vector.tensor_scalar(out=t1, in0=Sxy, scalar1=2.0 / 9.0,
                                    scalar2=C2 + C1, op0=Alu.mult, op1=Alu.add)
            num2 = tshort.tile([128, FD], bf16, tag="ts")
            nc.vector.tensor_sub(num2, t1, num1)
            num = tail.tile([128, FD], bf16, tag="num")
            nc.vector.tensor_mul(num, num1, num2)
            dd = trec.tile([128, FD], f32, tag="dd")
            nc.vector.tensor_mul(dd, den1, den2)
            r9 = trec.tile([128, FD], f32, tag="r9")
            nc.vector.reciprocal_approx_fast(out=r9, in_=dd)
            r9b = tshort.tile([128, FD], bf16, tag="ts")
            nc.scalar.activation(r9b, r9, Act.Copy)
            sm = tshort.tile([128, FD], bf16, tag="ts")
            nc.vector.tensor_mul(sm, num, r9b)
            scr = tshort.tile([128, FD], bf16, tag="ts")
            nc.scalar.activation(scr, sm, Act.Copy,
                                 accum_out=ssacc[:, p:p + 1])

        red = accs.tile([128, 4], f32, tag="red")
        nc.vector.reduce_sum(red[:, 0:1], xxacc, axis=mybir.AxisListType.X)
        nc.vector.reduce_sum(red[:, 1:2], yyacc, axis=mybir.AxisListType.X)
        nc.vector.reduce_sum(red[:, 2:3], xyacc, axis=mybir.AxisListType.X)
        nc.vector.reduce_sum(red[:, 3:4], ssacc, axis=mybir.AxisListType.X)
        dma.dma_start(out=out_d, in_=red)

    nc.compile()
    return nc


def _band_host():
    b = np.zeros((257, 128), np.float32)
    for i in range(128):            # BAND_A: i-j in {0,1,2}
        for j in range(128):
            if i - j in (0, 1, 2):
                b[i, j] = 1.0
    for i in range(127):            # BAND_B: i-j in {-1,0,1}
        for j in range(128):
            if i - j in (-1, 0, 1):
                b[128 + i, j] = 1.0
    b[255, 126] = 1.0               # E2 row 0: next-tile row 128c+127
    b[255, 127] = 1.0
    b[256, 127] = 1.0               # E2 row 1 / E1: row 128c+128
    return b


def _get_compiled():
    global _compiled
    if _compiled is None:
        _compiled = _build_nc()
    return _compiled


def _shard_inputs(reconstruction, target):
    import ml_dtypes
    dt = ml_dtypes.bfloat16
    band = _band_host().astype(dt)
    rec = np.asarray(reconstruction).reshape(N_CORES, PLANES, IMG, IMG).astype(dt)
    tgt = np.asarray(target).reshape(N_CORES, PLANES, IMG, IMG).astype(dt)
    return [{"x": np.ascontiguousarray(rec[i]),
             "y": np.ascontiguousarray(tgt[i]),
             "band": band} for i in range(N_CORES)]


def _combine(results):
    sxx = syy = sxy = sss = 0.0
    for i in range(N_CORES):
        red = results[i]["out"].astype(np.float64)
        sxx += red[:, 0].sum()
        syy += red[:, 1].sum()
        sxy += red[:, 2].sum()
        sss += red[:, 3].sum()
    n = float(N_CORES * PLANES * IMG * IMG)
    mse = (sxx + syy - 2.0 * sxy) / n
    ssim_loss = 1.0 - 9.0 * sss / n
    return np.float32(0.8 * mse + 0.2 * ssim_loss)


def run(reconstruction, target, trace=False):
    from concourse.bass_utils import run_bass_kernel_spmd
    nc = _get_compiled()
    in_maps = _shard_inputs(reconstruction, target)
    res = run_bass_kernel_spmd(nc, in_maps, list(range(N_CORES)), trace=trace)
    return _combine(res.results), res


def kernel(reconstruction, target):
    out, _ = run(reconstruction, target, trace=False)
    return out


# revision 54
# speedup vs baseline: 1.0260x; 1.0260x over previous
"""Trainium2 Bass kernel for EnhancedReconstructionLoss (0.8*MSE + 0.2*SSIM-loss).

Sharding: pure data parallel. Batch 32 -> 8 cores x 4 images (12 planes of
512x512 each). Each core computes partial sums (sum x^2, sum y^2, sum x*y,
sum ssim_map*9); host combines into the scalar loss.

Per-core pipeline per 512x512 plane (inputs pre-cast to bf16 on host; all
reductions accumulate in fp32 on-chip):
  - load x,y as a [128, 5, 512] plane tensor of row-shifted tiles
    (tile t holds rows 128t-1..128t+126) so cross-tile vertical-filter edge
    matmuls only need base-partition-0 operands
  - xx=x^2, yy=y^2 (ScalarE, fp32 accum for MSE), xy=x*y (accum), zz=xx+yy
    computed on the whole plane at once
  - vertical 3-tap box filter via TensorE banded matmul -> PSUM (fp32)
  - PSUM->SBUF bf16 copy (hw allows only one PSUM operand per instruction),
    horizontal 3-tap via two shifted-AP adds (DVE, bf16 2x mode)
  - SSIM pointwise tail once per plane at FD=2048 using tensor_tensor (2x)
    and tensor_scalar (4x) ops with the 1/9 pool normalizations folded into
    constants; the host multiplies the ssim sum by 9 at the end
"""

import sys
import numpy as np

for _p in ("/opt/trn_rl_repo", "/root/.axon_site/_ro/trn_rl_repo"):
    if _p not in sys.path:
        sys.path.insert(0, _p)

N_CORES = 8
IMG = 512
PLANES = 12          # 4 images x 3 channels per core
# Tiles are shifted by -1 row: tile t = rows 128t-1..128t+126 (tile 0 only
# 127 rows, tile 4 only row 511). Cross-tile matmul edges then only ever
# need the FIRST rows of the next tile (base partition 0, a hw requirement).
TILE_ROWS = [(0, 127), (127, 255), (255, 383), (383, 511), (511, 512)]
NT = 5
NCHUNK = 4
C1 = 0.01 ** 2
C2 = 0.03 ** 2
EPS = 1e-8

CFG = {
    "dma_eng": "sync",
}

_compiled = None


def _build_nc():
    from contextlib import ExitStack
    import concourse.bass as bass
    import concourse.tile as tile
    from concourse import bacc, mybir

    f32 = mybir.dt.float32
    bf16 = mybir.dt.bfloat16
    Alu = mybir.AluOpType
    Act = mybir.ActivationFunctionType

    nc = bacc.Bacc("TRN2", target_bir_lowering=False, debug=False,
                   enable_asserts=True, num_devices=N_CORES)
    x_d = nc.dram_tensor("x", [PLANES, IMG, IMG], bf16, kind="ExternalInput").ap()
    y_d = nc.dram_tensor("y", [PLANES, IMG, IMG], bf16, kind="ExternalInput").ap()
    band_d = nc.dram_tensor("band", [257, 128], bf16, kind="ExternalInput").ap()
    out_d = nc.dram_tensor("out", [128, 4], f32, kind="ExternalOutput").ap()

    dma = getattr(nc, CFG["dma_eng"])

    with tile.TileContext(nc) as tc, ExitStack() as ctx:
        consts = ctx.enter_context(tc.tile_pool(name="consts", bufs=1))
        inp = ctx.enter_context(tc.tile_pool(name="inp", bufs=3))
        pre = ctx.enter_context(tc.tile_pool(name="pre", bufs=2))
        psum = ctx.enter_context(tc.tile_pool(name="psum", bufs=2, space="PSUM"))
        taps = ctx.enter_context(tc.tile_pool(name="taps", bufs=3))
        s2p = ctx.enter_context(tc.tile_pool(name="s2p", bufs=2))
        tail = ctx.enter_context(tc.tile_pool(name="tail", bufs=2))
        trec = ctx.enter_context(tc.tile_pool(name="trec", bufs=1))
        tshort = ctx.enter_context(tc.tile_pool(name="tshort", bufs=8))
        accs = ctx.enter_context(tc.tile_pool(name="accs", bufs=1))

        band_a = consts.tile([128, 128], bf16, tag="band_a")  # i-j in {0,1,2}
        dma.dma_start(out=band_a, in_=band_d[0:128, :])
        band_b = consts.tile([127, 128], bf16, tag="band_b")  # i-j in {-1,0,1}
        dma.dma_start(out=band_b, in_=band_d[128:255, :])
        e2 = consts.tile([2, 128], bf16, tag="e2")
        dma.dma_start(out=e2, in_=band_d[255:257, :])
        e1 = consts.tile([1, 128], bf16, tag="e1")
        dma.dma_start(out=e1, in_=band_d[256:257, :])

        nacc = 5 + 2 * (PLANES - 1)
        xxacc = accs.tile([128, nacc], f32, tag="xxacc")
        yyacc = accs.tile([128, nacc], f32, tag="yyacc")
        xyacc = accs.tile([128, nacc], f32, tag="xyacc")
        ssacc = accs.tile([128, PLANES], f32, tag="ssacc")
        for a in (xxacc, yyacc, xyacc, ssacc):
            nc.vector.memset(a, 0.0)

        def load_plane(dst, src_d, p):
            # tile 0: rows 0..126 at partitions 0..126
            dma.dma_start(out=dst[0:127, 0, :], in_=src_d[p, 0:127, :])
            # tiles 1..2: rows 127..382, partition p = row 128t-1+p
            mid = src_d[p, 127:383, :].rearrange("(t r) c -> r t c", r=128)
            dma.dma_start(out=dst[:, 1:3, :], in_=mid)
            # tile 3: rows 383..510
            dma.dma_start(out=dst[:, 3, :], in_=src_d[p, 383:511, :])
            # tile 4: row 511 at partition 0
            dma.dma_start(out=dst[0:1, 4, :], in_=src_d[p, 511:512, :])

        for p in range(PLANES):
            # ---- load plane + pre-pool pointwise on the whole plane ----
            xp = inp.tile([128, NT, IMG], bf16, tag="xp")
            yp = inp.tile([128, NT, IMG], bf16, tag="yp")
            if p < 3:  # = inp pool bufs: zero each slot's pad regions once
                # the pads (t0 partition 127, t4 partitions 1..127) are never
                # DMA'd and slot values persist across the pool rotation, so
                # derived tensors inherit exact zeros there; partition bases
                # must be 0/32/64/96, so cover 96..127 (DMA rewrites 96..126)
                for t_ in (xp, yp):
                    nc.vector.memset(t_[96:128, 0, :], 0.0)
                    nc.vector.memset(t_[:, 4, :], 0.0)
            load_plane(xp, x_d, p)
            load_plane(yp, y_d, p)

            # pre-pool in two tile-halves so chunk 0/1 matmuls can start
            # before the last tiles of the plane have landed
            xxp = pre.tile([128, NT, IMG], bf16, tag="xx")
            yyp = pre.tile([128, NT, IMG], bf16, tag="yy")
            xyp = pre.tile([128, NT, IMG], bf16, tag="xy")
            zzp = pre.tile([128, NT, IMG], bf16, tag="zz")
            # plane 0 at per-tile granularity so compute starts as soon as
            # the first tile lands; later planes in two halves (less per-op
            # overhead, prefetch already hides the latency)
            if p == 0:
                hsplit = ((0, 1), (1, 2), (2, 3), (3, 4), (4, 5))
            else:
                hsplit = ((0, 3), (3, 5))
            for h, (t0, t1) in enumerate(hsplit):
                g = h if p == 0 else 5 + 2 * (p - 1) + h
                sl = (slice(None), slice(t0, t1), slice(None))
                nc.scalar.activation(xxp[sl], xp[sl], Act.Square,
                                     accum_out=xxacc[:, g:g + 1])
                nc.scalar.activation(yyp[sl], yp[sl], Act.Square,
                                     accum_out=yyacc[:, g:g + 1])
                nc.vector.scalar_tensor_tensor(
                    out=xyp[sl], in0=xp[sl], scalar=1.0, in1=yp[sl],
                    op0=Alu.mult, op1=Alu.mult, accum_out=xyacc[:, g:g + 1])
                nc.gpsimd.tensor_add(zzp[sl], xxp[sl], yyp[sl])

            streams = [xp, yp, zzp, xyp]

            # S2 holds the fully box-filtered sums for the whole plane:
            # [partition, stream, chunk, col]
            S2 = s2p.tile([128, 4, NCHUNK, IMG], bf16, tag="S2")

            # ---- per output chunk: vertical matmul + horizontal taps ----
            for c in range(NCHUNK):
                V = psum.tile([128, 4, IMG], f32, tag="V")
                for s, st in enumerate(streams):
                    main_band = band_b if c == 0 else band_a
                    main_rhs = st[0:127, 0, :] if c == 0 else st[:, c, :]
                    if c < NCHUNK - 1:
                        edge = (e2[0:2, :], st[0:2, c + 1, :])
                    else:
                        edge = (e1[0:1, :], st[0:1, c + 1, :])
                    mms = [(main_band, main_rhs), edge]
                    for i, (lhsT, rhs) in enumerate(mms):
                        nc.tensor.matmul(V[:, s, :], lhsT, rhs,
                                         start=(i == 0), stop=(i == len(mms) - 1))

                # Vs is 514 wide with persistent zero columns 0 and 513 (the
                # horizontal zero padding); the copy writes the middle 512
                Vs = taps.tile([128, 4, IMG + 2], bf16, tag="Vs")
                if p == 0 and c < 3:  # = taps pool bufs: zero pads once/slot
                    nc.vector.memset(Vs[:, :, 0:1], 0.0)
                    nc.vector.memset(Vs[:, :, IMG + 1:IMG + 2], 0.0)
                nc.scalar.activation(Vs[:, :, 1:IMG + 1], V, Act.Copy)

                A = taps.tile([128, 4, IMG], bf16, tag="A")
                nc.vector.tensor_add(A, Vs[:, :, 0:IMG],
                                     Vs[:, :, 2:IMG + 2])
                nc.vector.tensor_add(S2[:, :, c, :], A,
                                     Vs[:, :, 1:IMG + 1])

            # ---- SSIM pointwise tail, whole plane at once (FD = 2048) ----
            # With S = 9*mu (raw 3x3 box sums):
            #   num1 = 2*P/81 + C1            (P = Sx*Sy)
            #   num2 = 2*Sxy/9 + C2 - (num1 - C1)
            #   den1 = qsum/81 + C1           (qsum = Sx^2 + Sy^2)
            #   den2' = 9*den2 = Szz - (qsum/9 - 9*C2)
            #   ssim = num1*num2 / (den1*den2) = 9 * num / dd,  dd = den1*den2'
            # (the x9 is applied on the host)
            FD = NCHUNK * IMG
            Sx = S2[:, 0, :, :]
            Sy = S2[:, 1, :, :]
            Szz = S2[:, 2, :, :]
            Sxy = S2[:, 3, :, :]
            qx = tshort.tile([128, FD], bf16, tag="ts")
            nc.scalar.activation(qx, Sx, Act.Square)
            qy = tshort.tile([128, FD], bf16, tag="ts")
            nc.scalar.activation(qy, Sy, Act.Square)
            qsum = tshort.tile([128, FD], bf16, tag="ts")
            nc.vector.tensor_add(qsum, qx, qy)
            den1 = tail.tile([128, FD], bf16, tag="den1")
            nc.vector.tensor_scalar(out=den1, in0=qsum, scalar1=1.0 / 81.0,
                                    scalar2=C1, op0=Alu.mult, op1=Alu.add)
            U3 = tshort.tile([128, FD], bf16, tag="ts")
            nc.vector.tensor_scalar(out=U3, in0=qsum, scalar1=1.0 / 9.0,
                                    scalar2=-9.0 * C2, op0=Alu.mult, op1=Alu.add)
            den2 = tail.tile([128, FD], bf16, tag="den2")
            nc.vector.tensor_sub(den2, Szz, U3)
            P = tshort.tile([128, FD], bf16, tag="ts")
            nc.vector.tensor_mul(P, Sx, Sy)
            num1 = tail.tile([128, FD], bf16, tag="num1")
            nc.vector.tensor_scalar(out=num1, in0=P, scalar1=2.0 / 81.0,
                                    scalar2=C1, op0=Alu.mult, op1=Alu.add)
            t1 = tshort.tile([128, FD], bf16, tag="ts")
            nc.vector.tensor_scalar(out=t1, in0=Sxy, scalar1=2.0 / 9.0,
                                    scalar2=C2 + C1, op0=Alu.mult, op1=Alu.add)
            num2 = tshort.tile([128, FD], bf16, tag="ts")
            nc.vector.tensor_sub(num2, t1, num1)
            num = tail.tile([128, FD], bf16, tag="num")
            nc.vector.tensor_mul(num, num1, num2)
            dd = trec.tile([128, FD], f32, tag="dd")
            nc.vector.tensor_mul(dd, den1, den2)
            r9 = trec.tile([128, FD], f32, tag="r9")
            nc.vector.reciprocal_approx_fast(out=r9, in_=dd)
            r9b = tshort.tile([128, FD], bf16, tag="ts")
            nc.scalar.activation(r9b, r9, Act.Copy)
            sm = tshort.tile([128, FD], bf16, tag="ts")
            nc.vector.tensor_mul(sm, num, r9b)
            scr = tshort.tile([128, FD], bf16, tag="ts")
            nc.scalar.activation(scr, sm, Act.Copy,
                                 accum_out=ssacc[:, p:p + 1])

        red = accs.tile([128, 4], f32, tag="red")
        nc.vector.reduce_sum(red[:, 0:1], xxacc, axis=mybir.AxisListType.X)
        nc.vector.reduce_sum(red[:, 1:2], yyacc, axis=mybir.AxisListType.X)
        nc.vector.reduce_sum(red[:, 2:3], xyacc, axis=mybir.AxisListType.X)
        nc.vector.reduce_sum(red[:, 3:4], ssacc, axis=mybir.AxisListType.X)
        dma.dma_start(out=out_d, in_=red)

    nc.compile()
    return nc


def _band_host():
    b = np.zeros((257, 128), np.float32)
    for i in range(128):            # BAND_A: i-j in {0,1,2}
        for j in range(128):
            if i - j in (0, 1, 2):
                b[i, j] = 1.0
    for i in range(127):            # BAND_B: i-j in {-1,0,1}
        for j in range(128):
            if i - j in (-1, 0, 1):
                b[128 + i, j] = 1.0
    b[255, 126] = 1.0               # E2 row 0: next-tile row 128c+127
    b[255, 127] = 1.0
    b[256, 127] = 1.0               # E2 row 1 / E1: row 128c+128
    return b


def _get_compiled():
    global _compiled
    if _compiled is None:
        _compiled = _build_nc()
    return _compiled


def _shard_inputs(reconstruction, target):
    import ml_dtypes
    dt = ml_dtypes.bfloat16
    band = _band_host().astype(dt)
    rec = np.asarray(reconstruction).reshape(N_CORES, PLANES, IMG, IMG).astype(dt)
    tgt = np.asarray(target).reshape(N_CORES, PLANES, IMG, IMG).astype(dt)
    return [{"x": np.ascontiguousarray(rec[i]),
             "y": np.ascontiguousarray(tgt[i]),
             "band": band} for i in range(N_CORES)]


def _combine(results):
    sxx = syy = sxy = sss = 0.0
    for i in range(N_CORES):
        red = results[i]["out"].astype(np.float64)
        sxx += red[:, 0].sum()
        syy += red[:, 1].sum()
        sxy += red[:, 2].sum()
        sss += red[:, 3].sum()
    n = float(N_CORES * PLANES * IMG * IMG)
    mse = (sxx + syy - 2.0 * sxy) / n
    ssim_loss = 1.0 - 9.0 * sss / n
    return np.float32(0.8 * mse + 0.2 * ssim_loss)


def run(reconstruction, target, trace=False):
    from concourse.bass_utils import run_bass_kernel_spmd
    nc = _get_compiled()
    in_maps = _shard_inputs(reconstruction, target)
    res = run_bass_kernel_spmd(nc, in_maps, list(range(N_CORES)), trace=trace)
    return _combine(res.results), res


def kernel(reconstruction, target):
    out, _ = run(reconstruction, target, trace=False)
    return out


# revision 58
# speedup vs baseline: 1.0261x; 1.0001x over previous
"""Trainium2 Bass kernel for EnhancedReconstructionLoss (0.8*MSE + 0.2*SSIM-loss).

Sharding: pure data parallel. Batch 32 -> 8 cores x 4 images (12 planes of
512x512 each). Each core computes partial sums (sum x^2, sum y^2, sum x*y,
sum ssim_map*9); host combines into the scalar loss.

Per-core pipeline per 512x512 plane (inputs pre-cast to bf16 on host; all
reductions accumulate in fp32 on-chip):
  - load x,y as a [128, 5, 512] plane tensor of row-shifted tiles
    (tile t holds rows 128t-1..128t+126) so cross-tile vertical-filter edge
    matmuls only need base-partition-0 operands
  - xx=x^2, yy=y^2 (ScalarE, fp32 accum for MSE), xy=x*y (accum), zz=xx+yy
    computed on the whole plane at once
  - vertical 3-tap box filter via TensorE banded matmul -> PSUM (fp32)
  - PSUM->SBUF bf16 copy (hw allows only one PSUM operand per instruction),
    horizontal 3-tap via two shifted-AP adds (DVE, bf16 2x mode)
  - SSIM pointwise tail once per plane at FD=2048 using tensor_tensor (2x)
    and tensor_scalar (4x) ops with the 1/9 pool normalizations folded into
    constants; the host multiplies the ssim sum by 9 at the end
"""

import sys
import numpy as np

for _p in ("/opt/trn_rl_repo", "/root/.axon_site/_ro/trn_rl_repo"):
    if _p not in sys.path:
        sys.path.insert(0, _p)

N_CORES = 8
IMG = 512
PLANES = 12          # 4 images x 3 channels per core
# Tiles are shifted by -1 row: tile t = rows 128t-1..128t+126 (tile 0 only
# 127 rows, tile 4 only row 511). Cross-tile matmul edges then only ever
# need the FIRST rows of the next tile (base partition 0, a hw requirement).
TILE_ROWS = [(0, 127), (127, 255), (255, 383), (383, 511), (511, 512)]
NT = 5
NCHUNK = 4
C1 = 0.01 ** 2
C2 = 0.03 ** 2
EPS = 1e-8

CFG = {
    "dma_eng": "sync",
}

_compiled = None


def _build_nc():
    from contextlib import ExitStack
    import concourse.bass as bass
    import concourse.tile as tile
    from concourse import bacc, mybir

    f32 = mybir.dt.float32
    bf16 = mybir.dt.bfloat16
    Alu = mybir.AluOpType
    Act = mybir.ActivationFunctionType

    nc = bacc.Bacc("TRN2", target_bir_lowering=False, debug=False,
                   enable_asserts=True, num_devices=N_CORES)
    x_d = nc.dram_tensor("x", [PLANES, IMG, IMG], bf16, kind="ExternalInput").ap()
    y_d = nc.dram_tensor("y", [PLANES, IMG, IMG], bf16, kind="ExternalInput").ap()
    band_d = nc.dram_tensor("band", [257, 128], bf16, kind="ExternalInput").ap()
    out_d = nc.dram_tensor("out", [128, 4], f32, kind="ExternalOutput").ap()

    dma = getattr(nc, CFG["dma_eng"])

    with tile.TileContext(nc) as tc, ExitStack() as ctx:
        consts = ctx.enter_context(tc.tile_pool(name="consts", bufs=1))
        inp = ctx.enter_context(tc.tile_pool(name="inp", bufs=3))
        pre = ctx.enter_context(tc.tile_pool(name="pre", bufs=2))
        psum = ctx.enter_context(tc.tile_pool(name="psum", bufs=2, space="PSUM"))
        taps = ctx.enter_context(tc.tile_pool(name="taps", bufs=3))
        s2p = ctx.enter_context(tc.tile_pool(name="s2p", bufs=2))
        tail = ctx.enter_context(tc.tile_pool(name="tail", bufs=2))
        trec = ctx.enter_context(tc.tile_pool(name="trec", bufs=1))
        tshort = ctx.enter_context(tc.tile_pool(name="tshort", bufs=8))
        accs = ctx.enter_context(tc.tile_pool(name="accs", bufs=1))

        band_a = consts.tile([128, 128], bf16, tag="band_a")  # i-j in {0,1,2}
        dma.dma_start(out=band_a, in_=band_d[0:128, :])
        band_b = consts.tile([127, 128], bf16, tag="band_b")  # i-j in {-1,0,1}
        dma.dma_start(out=band_b, in_=band_d[128:255, :])
        e2 = consts.tile([2, 128], bf16, tag="e2")
        dma.dma_start(out=e2, in_=band_d[255:257, :])
        e1 = consts.tile([1, 128], bf16, tag="e1")
        dma.dma_start(out=e1, in_=band_d[256:257, :])

        nacc = 5 + 2 * (PLANES - 1)
        xxacc = accs.tile([128, nacc], f32, tag="xxacc")
        yyacc = accs.tile([128, nacc], f32, tag="yyacc")
        xyacc = accs.tile([128, nacc], f32, tag="xyacc")
        ssacc = accs.tile([128, PLANES + 1], f32, tag="ssacc")
        for a in (xxacc, yyacc, xyacc, ssacc):
            nc.vector.memset(a, 0.0)

        def load_plane(dst, src_d, p):
            # tile 0: rows 0..126 at partitions 0..126
            dma.dma_start(out=dst[0:127, 0, :], in_=src_d[p, 0:127, :])
            # tiles 1..2: rows 127..382, partition p = row 128t-1+p
            mid = src_d[p, 127:383, :].rearrange("(t r) c -> r t c", r=128)
            dma.dma_start(out=dst[:, 1:3, :], in_=mid)
            # tile 3: rows 383..510
            dma.dma_start(out=dst[:, 3, :], in_=src_d[p, 383:511, :])
            # tile 4: row 511 at partition 0
            dma.dma_start(out=dst[0:1, 4, :], in_=src_d[p, 511:512, :])

        def tail_body(S2, c0, c1, g):
            FD = (c1 - c0) * IMG
            Sx = S2[:, 0, c0:c1, :]
            Sy = S2[:, 1, c0:c1, :]
            Szz = S2[:, 2, c0:c1, :]
            Sxy = S2[:, 3, c0:c1, :]
            qx = tshort.tile([128, FD], bf16, tag="ts")
            nc.scalar.activation(qx, Sx, Act.Square)
            qy = tshort.tile([128, FD], bf16, tag="ts")
            nc.scalar.activation(qy, Sy, Act.Square)
            qsum = tshort.tile([128, FD], bf16, tag="ts")
            nc.vector.tensor_add(qsum, qx, qy)
            den1 = tail.tile([128, FD], bf16, tag="den1")
            nc.vector.tensor_scalar(out=den1, in0=qsum, scalar1=1.0 / 81.0,
                                    scalar2=C1, op0=Alu.mult, op1=Alu.add)
            U3 = tshort.tile([128, FD], bf16, tag="ts")
            nc.vector.tensor_scalar(out=U3, in0=qsum, scalar1=1.0 / 9.0,
                                    scalar2=-9.0 * C2, op0=Alu.mult, op1=Alu.add)
            den2 = tail.tile([128, FD], bf16, tag="den2")
            nc.vector.tensor_sub(den2, Szz, U3)
            P = tshort.tile([128, FD], bf16, tag="ts")
            nc.vector.tensor_mul(P, Sx, Sy)
            num1 = tail.tile([128, FD], bf16, tag="num1")
            nc.vector.tensor_scalar(out=num1, in0=P, scalar1=2.0 / 81.0,
                                    scalar2=C1, op0=Alu.mult, op1=Alu.add)
            t1 = tshort.tile([128, FD], bf16, tag="ts")
            nc.vector.tensor_scalar(out=t1, in0=Sxy, scalar1=2.0 / 9.0,
                                    scalar2=C2 + C1, op0=Alu.mult, op1=Alu.add)
            num2 = tshort.tile([128, FD], bf16, tag="ts")
            nc.vector.tensor_sub(num2, t1, num1)
            num = tail.tile([128, FD], bf16, tag="num")
            nc.vector.tensor_mul(num, num1, num2)
            dd = trec.tile([128, FD], f32, tag="dd")
            nc.vector.tensor_mul(dd, den1, den2)
            r9 = trec.tile([128, FD], f32, tag="r9")
            nc.vector.reciprocal_approx_fast(out=r9, in_=dd)
            r9b = tshort.tile([128, FD], bf16, tag="ts")
            nc.scalar.activation(r9b, r9, Act.Copy)
            sm = tshort.tile([128, FD], bf16, tag="ts")
            nc.vector.tensor_mul(sm, num, r9b)
            scr = tshort.tile([128, FD], bf16, tag="ts")
            nc.scalar.activation(scr, sm, Act.Copy,
                                 accum_out=ssacc[:, g:g + 1])

        for p in range(PLANES):
            # ---- load plane + pre-pool pointwise on the whole plane ----
            xp = inp.tile([128, NT, IMG], bf16, tag="xp")
            yp = inp.tile([128, NT, IMG], bf16, tag="yp")
            if p < 3:  # = inp pool bufs: zero each slot's pad regions once
                # the pads (t0 partition 127, t4 partitions 1..127) are never
                # DMA'd and slot values persist across the pool rotation, so
                # derived tensors inherit exact zeros there; partition bases
                # must be 0/32/64/96, so cover 96..127 (DMA rewrites 96..126)
                for t_ in (xp, yp):
                    nc.vector.memset(t_[96:128, 0, :], 0.0)
                    nc.vector.memset(t_[:, 4, :], 0.0)
            load_plane(xp, x_d, p)
            load_plane(yp, y_d, p)

            # pre-pool in two tile-halves so chunk 0/1 matmuls can start
            # before the last tiles of the plane have landed
            xxp = pre.tile([128, NT, IMG], bf16, tag="xx")
            yyp = pre.tile([128, NT, IMG], bf16, tag="yy")
            xyp = pre.tile([128, NT, IMG], bf16, tag="xy")
            zzp = pre.tile([128, NT, IMG], bf16, tag="zz")
            # plane 0 at per-tile granularity so compute starts as soon as
            # the first tile lands; later planes in two halves (less per-op
            # overhead, prefetch already hides the latency)
            if p == 0:
                hsplit = ((0, 1), (1, 2), (2, 3), (3, 4), (4, 5))
            else:
                hsplit = ((0, 3), (3, 5))
            for h, (t0, t1) in enumerate(hsplit):
                g = h if p == 0 else 5 + 2 * (p - 1) + h
                sl = (slice(None), slice(t0, t1), slice(None))
                nc.scalar.activation(xxp[sl], xp[sl], Act.Square,
                                     accum_out=xxacc[:, g:g + 1])
                nc.scalar.activation(yyp[sl], yp[sl], Act.Square,
                                     accum_out=yyacc[:, g:g + 1])
                nc.vector.scalar_tensor_tensor(
                    out=xyp[sl], in0=xp[sl], scalar=1.0, in1=yp[sl],
                    op0=Alu.mult, op1=Alu.mult, accum_out=xyacc[:, g:g + 1])
                nc.gpsimd.tensor_add(zzp[sl], xxp[sl], yyp[sl])

            streams = [xp, yp, zzp, xyp]

            # S2 holds the fully box-filtered sums for the whole plane:
            # [partition, stream, chunk, col]
            S2 = s2p.tile([128, 4, NCHUNK, IMG], bf16, tag="S2")

            # ---- per output chunk: vertical matmul + horizontal taps ----
            for c in range(NCHUNK):
                V = psum.tile([128, 4, IMG], f32, tag="V")
                for s, st in enumerate(streams):
                    main_band = band_b if c == 0 else band_a
                    main_rhs = st[0:127, 0, :] if c == 0 else st[:, c, :]
                    if c < NCHUNK - 1:
                        edge = (e2[0:2, :], st[0:2, c + 1, :])
                    else:
                        edge = (e1[0:1, :], st[0:1, c + 1, :])
                    mms = [(main_band, main_rhs), edge]
                    for i, (lhsT, rhs) in enumerate(mms):
                        nc.tensor.matmul(V[:, s, :], lhsT, rhs,
                                         start=(i == 0), stop=(i == len(mms) - 1))

                # Vs is 514 wide with persistent zero columns 0 and 513 (the
                # horizontal zero padding); the copy writes the middle 512
                Vs = taps.tile([128, 4, IMG + 2], bf16, tag="Vs")
                if p == 0 and c < 3:  # = taps pool bufs: zero pads once/slot
                    nc.vector.memset(Vs[:, :, 0:1], 0.0)
                    nc.vector.memset(Vs[:, :, IMG + 1:IMG + 2], 0.0)
                nc.scalar.activation(Vs[:, :, 1:IMG + 1], V, Act.Copy)

                A = taps.tile([128, 4, IMG], bf16, tag="A")
                nc.vector.tensor_add(A, Vs[:, :, 0:IMG],
                                     Vs[:, :, 2:IMG + 2])
                nc.vector.tensor_add(S2[:, :, c, :], A,
                                     Vs[:, :, 1:IMG + 1])

            # ---- SSIM pointwise tail, whole plane at once (FD = 2048) ----
            # With S = 9*mu (raw 3x3 box sums):
            #   num1 = 2*P/81 + C1            (P = Sx*Sy)
            #   num2 = 2*Sxy/9 + C2 - (num1 - C1)
            #   den1 = qsum/81 + C1           (qsum = Sx^2 + Sy^2)
            #   den2' = 9*den2 = Szz - (qsum/9 - 9*C2)
            #   ssim = num1*num2 / (den1*den2) = 9 * num / dd,  dd = den1*den2'
            # (the x9 is applied on the host)
            # the last plane's tail has no successor work to hide its serial
            # chain, so run it as two half-width passes that pipeline
            if p == PLANES - 1:
                tail_parts = [(0, 2, p), (2, 4, p + 1)]
            else:
                tail_parts = [(0, NCHUNK, p)]
            for c0, c1, g in tail_parts:
                tail_body(S2, c0, c1, g)

        red = accs.tile([128, 4], f32, tag="red")
        nc.vector.reduce_sum(red[:, 0:1], xxacc, axis=mybir.AxisListType.X)
        nc.vector.reduce_sum(red[:, 1:2], yyacc, axis=mybir.AxisListType.X)
        nc.vector.reduce_sum(red[:, 2:3], xyacc, axis=mybir.AxisListType.X)
        nc.vector.reduce_sum(red[:, 3:4], ssacc, axis=mybir.AxisListType.X)
        dma.dma_start(out=out_d, in_=red)

    nc.compile()
    return nc




def _band_host():
    b = np.zeros((257, 128), np.float32)
    for i in range(128):            # BAND_A: i-j in {0,1,2}
        for j in range(128):
            if i - j in (0, 1, 2):
                b[i, j] = 1.0
    for i in range(127):            # BAND_B: i-j in {-1,0,1}
        for j in range(128):
            if i - j in (-1, 0, 1):
                b[128 + i, j] = 1.0
    b[255, 126] = 1.0               # E2 row 0: next-tile row 128c+127
    b[255, 127] = 1.0
    b[256, 127] = 1.0               # E2 row 1 / E1: row 128c+128
    return b


def _get_compiled():
    global _compiled
    if _compiled is None:
        _compiled = _build_nc()
    return _compiled


def _shard_inputs(reconstruction, target):
    import ml_dtypes
    dt = ml_dtypes.bfloat16
    band = _band_host().astype(dt)
    rec = np.asarray(reconstruction).reshape(N_CORES, PLANES, IMG, IMG).astype(dt)
    tgt = np.asarray(target).reshape(N_CORES, PLANES, IMG, IMG).astype(dt)
    return [{"x": np.ascontiguousarray(rec[i]),
             "y": np.ascontiguousarray(tgt[i]),
             "band": band} for i in range(N_CORES)]


def _combine(results):
    sxx = syy = sxy = sss = 0.0
    for i in range(N_CORES):
        red = results[i]["out"].astype(np.float64)
        sxx += red[:, 0].sum()
        syy += red[:, 1].sum()
        sxy += red[:, 2].sum()
        sss += red[:, 3].sum()
    n = float(N_CORES * PLANES * IMG * IMG)
    mse = (sxx + syy - 2.0 * sxy) / n
    ssim_loss = 1.0 - 9.0 * sss / n
    return np.float32(0.8 * mse + 0.2 * ssim_loss)


def run(reconstruction, target, trace=False):
    from concourse.bass_utils import run_bass_kernel_spmd
    nc = _get_compiled()
    in_maps = _shard_inputs(reconstruction, target)
    res = run_bass_kernel_spmd(nc, in_maps, list(range(N_CORES)), trace=trace)
    return _combine(res.results), res


def kernel(reconstruction, target):
    out, _ = run(reconstruction, target, trace=False)
    return out
